# revision 10
# baseline (speedup 1.0000x reference)
"""Trainium2 Bass kernel for the KKT loss (nn_KKTLoss_46299747451217).

Strategy (8 NeuronCores, SPMD), v2 — fp8 DoubleRow:
  - All matmul operands are fp8 e4m3 (PE DoubleRow mode: 2 contraction rows
    per cycle, fp32 PSUM accumulation). Power-of-2 scales keep values in the
    e4m3 normal range: activations x4, grid matrices x64; the compensation
    (exact powers of two) is folded into the element-wise multipliers and
    activation scale factors.
  - Y and Yconj are folded into S = Y + Yconj host-side (the reference only
    ever uses Y-rows and Yconj-rows through the same quadratic form).
  - Row-sharding as v1: S 250 rows/core (+ row n+1), IM 750 rows/core,
    Ybr 375 real + 375 imag line rows/core, Map_g 500 rows/core.
  - Stage 1 computes T^T directly ([t-rows, batch] PSUM tiles, IM-stationary
    matmuls) so no PE transposes are needed; T^T is stored fp8 and
    AllGathered (1.5 MB instead of 3.1 MB).
  - DMA priority: the sync queue carries only the stage-1-critical vt/imt
    chunks; the scalar queue (blob + S/at/mapt) and vector queue (ybrt
    prefetch) are gated on early imt chunks so they don't steal HBM
    bandwidth from stage 1. The ttg read-back is queued on gpsimd directly
    behind the collective.
  - Element-wise penalties in bf16 exactly as v1 (proven 5e-4 accurate);
    fp8 matmul error verified ~1e-2 end-to-end vs the fp32 reference.
  - Each core outputs a partial [256] loss; the host sums the 8 partials and
    adds the tiny slack/pq terms.
"""

import os
import numpy as np
import ml_dtypes

import concourse.bass as bass
import concourse.bacc as bacc
import concourse.mybir as mybir
import concourse.tile as tile
from concourse.bass_utils import run_bass_kernel_spmd

F32 = mybir.dt.float32
BF16 = mybir.dt.bfloat16
FP8 = mybir.dt.float8e4
ALU = mybir.AluOpType
ACTF = mybir.ActivationFunctionType
DR = mybir.MatmulPerfMode.DoubleRow

# ---------------------------------------------------------------- constants
B = 256            # batch
N = 2000           # n_bus
NL = 3000          # n_line
NCORE = 8
KT4, DKT4 = 32, 16   # k tiles / double-k tiles over padded 2n = 4096
KT6, DKT6 = 48, 24   # k tiles over gathered-T contraction (8 * 768)
YROW = 250         # S rows per core
TROW, TPAD = 750, 768
MROW, MPAD = 500, 512
LROW, LPAD = 375, 384
VROW, VPAD = 250, 256
NPs = 12           # positive accumulator slots per b-tile
NNs = 8            # negative accumulator slots per b-tile

SA = 4.0           # activation fp8 scale (Volt, n_o_l_p*Lg0)
SW = 64.0          # matrix fp8 scale (S, IM, Ybr, Map_g)
# stage1 psum = SA*SW*T -> store T*1; stage2 psum = SW*Ibr; Y psum = SA*SW*SV
INV_AW = 1.0 / (SA * SW)   # 2^-8
INV_W = 1.0 / SW           # 2^-6

# blob layout: name -> (offset, width), all bf16, [128, _BLOBW]
_BLOB_SPEC = [
    ("mult", 512), ("pqg", 1024), ("mgu", 1024), ("mgd", 1024),
    ("cpq", 1024), ("vr", 512), ("vi", 512), ("mvu", 512), ("mvd", 512),
    ("miu", 768), ("gmaxr", 512), ("gminr", 512), ("vmax2r", 256),
    ("vmin2r", 256), ("l2r", 384),
]
_BLOB_OFF = {}
_off = 0
for _nm, _w in _BLOB_SPEC:
    _BLOB_OFF[_nm] = (_off, _w)
    _off += _w
_BLOBW = _off

_CACHE = {}


# ---------------------------------------------------------------- builders
def _build_nc():
    nc = bacc.Bacc("TRN2", target_bir_lowering=False, debug=False,
                   num_devices=NCORE)

    # fp8 k-tile-packed matrices: [128, KT*C] with column block per k-tile
    d_vt = nc.dram_tensor("vt", [128, KT4 * 256], FP8, kind="ExternalInput")
    d_at = nc.dram_tensor("at", [128, KT4 * 256], FP8, kind="ExternalInput")
    d_yy = nc.dram_tensor("yy", [128, KT4 * 256], FP8, kind="ExternalInput")
    d_imt = nc.dram_tensor("imt", [128, KT4 * TPAD], FP8, kind="ExternalInput")
    d_mapt = nc.dram_tensor("mapt", [128, KT4 * MPAD], FP8,
                            kind="ExternalInput")
    d_ybrt = nc.dram_tensor("ybrt", [128, KT6 * TPAD], FP8,
                            kind="ExternalInput")
    d_blob = nc.dram_tensor("blob", [128, _BLOBW], BF16, kind="ExternalInput")
    # per-partition scalar columns: [256*Lg1, 256*Lg2, 1/n_gbus]
    d_cols = nc.dram_tensor("cols", [128, 3], F32, kind="ExternalInput")
    d_out = nc.dram_tensor("out", [128, 2], F32, kind="ExternalOutput")

    with tile.TileContext(nc) as tc:
        with (
            tc.tile_pool(name="res", bufs=1) as res,
            tc.tile_pool(name="scr", bufs=4) as scr,
            tc.tile_pool(name="ps", bufs=8, space="PSUM") as ps,
            tc.tile_pool(name="dram", bufs=1, space="DRAM") as dram,
        ):
            # ---- stage-1-critical loads on the sync queue, interleaved so
            # the first double-k-tile matmuls unblock after ~1 chunk
            vt = res.tile([128, KT4, 256], FP8)
            imt = res.tile([128, KT4, TPAD], FP8)
            vt2 = vt.rearrange("p k c -> p (k c)")
            imt2 = imt.rearrange("p k c -> p (k c)")
            for j in range(8):
                if j % 2 == 0:
                    jv = j // 2
                    nc.sync.dma_start(vt2[:, jv * 2048:(jv + 1) * 2048],
                                      d_vt[:, jv * 2048:(jv + 1) * 2048])
                nc.sync.dma_start(
                    imt2[:, j * 4 * TPAD:(j + 1) * 4 * TPAD],
                    d_imt[:, j * 4 * TPAD:(j + 1) * 4 * TPAD])

            # ---- stage-2 weights prefetch: behind the stage-1 loads on the
            # same in-order sync ring (starts ~12us, full spare bandwidth)
            ybr = res.tile([128, KT6, TPAD], FP8)
            ybr2 = ybr.rearrange("p k c -> p (k c)")
            for j in range(6):
                nc.sync.dma_start(
                    ybr2[:, j * 8 * TPAD:(j + 1) * 8 * TPAD],
                    d_ybrt[:, j * 8 * TPAD:(j + 1) * 8 * TPAD])

            # ---- scalar queue: gate on imt chunk 5 so the stage-1 loads get
            # nearly full HBM bandwidth first, then load the Y/Map/penalty
            # inputs
            gate_s = res.tile([128, 2], F32)
            nc.scalar.activation(gate_s[0:1, 0:1],
                                 imt2[0:1, 23 * TPAD + 767:23 * TPAD + 768],
                                 ACTF.Copy)
            blob = res.tile([128, _BLOBW], BF16)
            blob_h = _BLOBW // 2
            nc.scalar.dma_start(blob[:, :blob_h], d_blob[:, :blob_h])
            nc.scalar.dma_start(blob[:, blob_h:], d_blob[:, blob_h:])
            cols = res.tile([128, 3], F32)
            nc.scalar.dma_start(cols[:], d_cols[:])
            yy = res.tile([128, KT4, 256], FP8)
            nc.scalar.dma_start(yy.rearrange("p k c -> p (k c)")[:], d_yy[:])
            at = res.tile([128, KT4, 256], FP8)
            nc.scalar.dma_start(at.rearrange("p k c -> p (k c)")[:], d_at[:])
            mapt = res.tile([128, KT4, MPAD], FP8)
            mapt2 = mapt.rearrange("p k c -> p (k c)")
            nc.scalar.dma_start(mapt2[:, :KT4 * MPAD // 2],
                                d_mapt[:, :KT4 * MPAD // 2])
            nc.scalar.dma_start(mapt2[:, KT4 * MPAD // 2:],
                                d_mapt[:, KT4 * MPAD // 2:])

            small = {nm: blob[:, o:o + w] for nm, (o, w) in _BLOB_OFF.items()}
            lg1 = cols[:, 0:1]
            lg2 = cols[:, 1:2]
            ngbinv = cols[:, 2:3]

            # ---- PE warm-up: dummy matmuls on a zeroed tile ramp the tensor
            # engine's p-state to full clock while the stage-1 DMAs land
            warm = res.tile([128, 2, 256], FP8)
            nc.vector.memset(warm.rearrange("p a b -> p (a b)")[:], 0.0)
            wps = ps.tile([128, 512], F32, tag="mm", name="warm_ps")
            for _ in range(48):
                nc.tensor.matmul(
                    wps[:, :256], warm[:, :, 0:128], warm[:],
                    start=True, stop=True, perf_mode=DR,
                    skip_group_check=True,
                )

            # ---- stage 1: T^T tiles [t(128), batch(256)], IM-stationary
            ps1 = [ps.tile([128, 512], F32, tag="mm", name=f"ps1_{tb}")
                   for tb in range(6)]
            for dk in range(DKT4):
                for tb in range(6):
                    nc.tensor.matmul(
                        ps1[tb][:, :256],
                        imt[:, 2 * dk:2 * dk + 2, tb * 128:(tb + 1) * 128],
                        vt[:, 2 * dk:2 * dk + 2, :],
                        start=(dk == 0), stop=(dk == DKT4 - 1),
                        perf_mode=DR,
                    )
            # drain to fp8 (T_true = psum / (SA*SW)); halves A (t 0-383) and
            # B (t 384-767) feed two pipelined AllGathers
            tt_dram = [dram.tile([3 * 128, 256], FP8, name=f"tt{h}")
                       for h in range(2)]
            tsb = []
            for tb in range(6):
                t_sb = scr.tile([128, 256], FP8, tag="tsb", name=f"tsb{tb}")
                nc.scalar.activation(t_sb[:], ps1[tb][:, :256], ACTF.Copy,
                                     scale=INV_AW)
                tsb.append(t_sb)
            for tb in range(3):
                nc.gpsimd.dma_start(tt_dram[0][tb * 128:(tb + 1) * 128, :],
                                    tsb[tb][:])
            for tb in range(3):
                # B-half stores ride the sync ring (behind ybrt) so they don't
                # delay collective A on the gpsimd ring
                nc.sync.dma_start(tt_dram[1][tb * 128:(tb + 1) * 128, :],
                                  tsb[3 + tb][:])

            ttg_dram = [dram.tile([24 * 128, 256], FP8, addr_space="Shared",
                                  name=f"ttg{h}") for h in range(2)]
            for h in range(2):
                nc.gpsimd.collective_compute(
                    "AllGather",
                    ALU.bypass,
                    replica_groups=[list(range(NCORE))],
                    ins=[tt_dram[h].opt()],
                    outs=[ttg_dram[h].opt()],
                )
            # read back the gathered T^T halves, k-tiled, on the scalar ring
            # (data-dependent on each collective; overlaps the next one)
            ttg = res.tile([128, KT6, 256], FP8)
            for h in range(2):
                for j in range(2):
                    tv = ttg_dram[h][j * 1536:(j + 1) * 1536, :].rearrange(
                        "(k p) b -> p k b", p=128)
                    nc.scalar.dma_start(
                        ttg[:, h * 24 + j * 12:h * 24 + (j + 1) * 12, :], tv)

            # accumulator strips
            accp = res.tile([128, 2, NPs], F32)
            accn = res.tile([128, 2, NNs], F32)
            nc.vector.memset(accp[:], 0.0)
            nc.vector.memset(accn[:], 0.0)
            ip = [0, 0]
            iq = [0, 0]

            def slot_p(bt):
                j = ip[bt]
                ip[bt] += 1
                assert j < NPs
                return accp[:, bt, j:j + 1]

            def slot_n(bt):
                j = iq[bt]
                iq[bt] += 1
                assert j < NNs
                return accn[:, bt, j:j + 1]

            # ---- S = Y+Yconj quadratic term (psum = SA*SW*(S V);
            # mult blob is pre-divided by SA*SW)
            for bt in range(2):
                q = ps.tile([128, 512], F32, tag="mm", name=f"q{bt}")
                for dk in range(DKT4):
                    nc.tensor.matmul(
                        q[:, :256],
                        vt[:, 2 * dk:2 * dk + 2, bt * 128:(bt + 1) * 128],
                        yy[:, 2 * dk:2 * dk + 2, :],
                        start=(dk == 0), stop=(dk == DKT4 - 1),
                        perf_mode=DR,
                    )
                oq = scr.tile([128, 256], F32, tag="s256y", name=f"oq{bt}")
                nc.vector.tensor_tensor(
                    out=oq[:], in0=q[:, :256],
                    in1=small["mult"][:, bt * 256:(bt + 1) * 256], op=ALU.mult)
                nc.vector.reduce_sum(out=slot_p(bt), in_=oq[:],
                                     axis=mybir.AxisListType.X)

            # ---- Map_g dual/stationarity term (psum = SA*SW*(a Map^T);
            # lg1/lg2/cpq are pre-scaled by SA*SW, final Abs scales back)
            for bt in range(2):
                d = ps.tile([128, 512], F32, tag="mm", name=f"d{bt}")
                for dk in range(DKT4):
                    nc.tensor.matmul(
                        d[:],
                        at[:, 2 * dk:2 * dk + 2, bt * 128:(bt + 1) * 128],
                        mapt[:, 2 * dk:2 * dk + 2, :],
                        start=(dk == 0), stop=(dk == DKT4 - 1),
                        perf_mode=DR,
                    )
                sl = slice(bt * 512, (bt + 1) * 512)
                t1 = scr.tile([128, 512], F32, tag="s512", name=f"du1_{bt}")
                nc.vector.scalar_tensor_tensor(
                    out=t1[:], in0=small["mgu"][:, sl], scalar=lg1, in1=d[:],
                    op0=ALU.mult, op1=ALU.add)
                t2 = scr.tile([128, 512], F32, tag="s512", name=f"du2_{bt}")
                nc.vector.scalar_tensor_tensor(
                    out=t2[:], in0=small["mgd"][:, sl], scalar=lg2, in1=t1[:],
                    op0=ALU.mult, op1=ALU.subtract)
                t3 = scr.tile([128, 512], F32, tag="s512", name=f"du3_{bt}")
                nc.vector.tensor_tensor(
                    out=t3[:], in0=t2[:], in1=small["cpq"][:, sl], op=ALU.add)
                t4 = scr.tile([128, 512], F32, tag="s512", name=f"du4_{bt}")
                nc.scalar.activation(t4[:], t3[:], ACTF.Abs, scale=INV_AW,
                                     accum_out=slot_p(bt))

            # ---- generator limit + complementary slackness terms
            for bt in range(2):
                sl = slice(bt * 512, (bt + 1) * 512)
                d1 = scr.tile([128, 512], F32, tag="s512", name=f"g1_{bt}")
                nc.vector.tensor_tensor(out=d1[:], in0=small["pqg"][:, sl],
                                        in1=small["gmaxr"][:], op=ALU.subtract)
                r1 = scr.tile([128, 512], F32, tag="s512", name=f"g2_{bt}")
                nc.vector.tensor_scalar(out=r1[:], in0=d1[:], scalar1=0.0,
                                        scalar2=None, op0=ALU.max,
                                        op1=ALU.add, accum_out=slot_p(bt))
                m1 = scr.tile([128, 512], F32, tag="s512", name=f"g3_{bt}")
                nc.vector.tensor_tensor(out=m1[:], in0=d1[:],
                                        in1=small["mgu"][:, sl], op=ALU.mult)
                a1 = scr.tile([128, 512], F32, tag="s512", name=f"g4_{bt}")
                nc.scalar.activation(a1[:], m1[:], ACTF.Abs, scale=ngbinv,
                                     accum_out=slot_p(bt))

                d2 = scr.tile([128, 512], F32, tag="s512", name=f"g5_{bt}")
                nc.vector.tensor_tensor(out=d2[:], in0=small["pqg"][:, sl],
                                        in1=small["gminr"][:], op=ALU.subtract)
                r2 = scr.tile([128, 512], F32, tag="s512", name=f"g6_{bt}")
                nc.vector.tensor_scalar(out=r2[:], in0=d2[:], scalar1=0.0,
                                        scalar2=None, op0=ALU.min,
                                        op1=ALU.add, accum_out=slot_n(bt))
                m2 = scr.tile([128, 512], F32, tag="s512", name=f"g7_{bt}")
                nc.vector.tensor_tensor(out=m2[:], in0=d2[:],
                                        in1=small["mgd"][:, sl], op=ALU.mult)
                a2 = scr.tile([128, 512], F32, tag="s512", name=f"g8_{bt}")
                nc.scalar.activation(a2[:], m2[:], ACTF.Abs, scale=ngbinv,
                                     accum_out=slot_p(bt))

            # ---- voltage magnitude terms
            for bt in range(2):
                sl = slice(bt * VPAD, (bt + 1) * VPAD)
                s1 = scr.tile([128, VPAD], F32, tag="s256", name=f"v1_{bt}")
                nc.scalar.activation(s1[:], small["vr"][:, sl], ACTF.Square)
                s2 = scr.tile([128, VPAD], F32, tag="s256", name=f"v2_{bt}")
                nc.scalar.activation(s2[:], small["vi"][:, sl], ACTF.Square)
                msq = scr.tile([128, VPAD], F32, tag="s256", name=f"v3_{bt}")
                nc.vector.tensor_tensor(out=msq[:], in0=s1[:], in1=s2[:],
                                        op=ALU.add)
                dv1 = scr.tile([128, VPAD], F32, tag="s256", name=f"v4_{bt}")
                nc.vector.tensor_tensor(out=dv1[:], in0=msq[:],
                                        in1=small["vmax2r"][:], op=ALU.subtract)
                rv1 = scr.tile([128, VPAD], F32, tag="s256", name=f"v5_{bt}")
                nc.vector.tensor_scalar(out=rv1[:], in0=dv1[:], scalar1=0.0,
                                        scalar2=None, op0=ALU.max,
                                        op1=ALU.add, accum_out=slot_p(bt))
                mv1 = scr.tile([128, VPAD], F32, tag="s256", name=f"v6_{bt}")
                nc.vector.tensor_tensor(out=mv1[:], in0=dv1[:],
                                        in1=small["mvu"][:, sl], op=ALU.mult)
                av1 = scr.tile([128, VPAD], F32, tag="s256", name=f"v7_{bt}")
                nc.scalar.activation(av1[:], mv1[:], ACTF.Abs,
                                     accum_out=slot_p(bt))
                dv2 = scr.tile([128, VPAD], F32, tag="s256", name=f"v8_{bt}")
                nc.vector.tensor_tensor(out=dv2[:], in0=msq[:],
                                        in1=small["vmin2r"][:], op=ALU.subtract)
                rv2 = scr.tile([128, VPAD], F32, tag="s256", name=f"v9_{bt}")
                nc.vector.tensor_scalar(out=rv2[:], in0=dv2[:], scalar1=0.0,
                                        scalar2=None, op0=ALU.min,
                                        op1=ALU.add, accum_out=slot_n(bt))
                mv2 = scr.tile([128, VPAD], F32, tag="s256", name=f"va_{bt}")
                nc.vector.tensor_tensor(out=mv2[:], in0=dv2[:],
                                        in1=small["mvd"][:, sl], op=ALU.mult)
                av2 = scr.tile([128, VPAD], F32, tag="s256", name=f"vb_{bt}")
                nc.scalar.activation(av2[:], mv2[:], ACTF.Abs,
                                     accum_out=slot_p(bt))

            # ---- dual feasibility: sum relu(-mu) == -sum min(mu, 0)
            for bt in range(2):
                for nm, w in (("mgu", 512), ("mgd", 512), ("mvu", VPAD),
                              ("mvd", VPAD), ("miu", LPAD)):
                    sl = slice(bt * w, (bt + 1) * w)
                    f = scr.tile([128, w], F32, tag=f"s{w}",
                                 name=f"f_{nm}_{bt}")
                    nc.vector.tensor_scalar(out=f[:], in0=small[nm][:, sl],
                                            scalar1=0.0, scalar2=None,
                                            op0=ALU.min, op1=ALU.add,
                                            accum_out=slot_n(bt))

            # ---- stage 2: branch currents (psum = SW*Ibr; squares are
            # rescaled by 1/SW inside the Square activation). Phase order
            # bt0-A, bt1-A, bt0-B, bt1-B keeps the tensor engine busy on
            # A-half matmuls while gather B is still in flight, and lets
            # bt0's penalty chain overlap bt1's B-half matmuls.
            ps2 = [[ps.tile([128, 512], F32, name=f"ps2_{bt}_{ch}", tag="mm")
                    for ch in range(2)] for bt in range(2)]
            for h, bt in ((0, 0), (0, 1), (1, 0), (1, 1)):
                for dk in range(DKT6 // 2):
                    kk = h * 24 + 2 * dk
                    for ch in range(2):
                        nc.tensor.matmul(
                            ps2[bt][ch][:, :LPAD],
                            ttg[:, kk:kk + 2, bt * 128:(bt + 1) * 128],
                            ybr[:, kk:kk + 2, ch * LPAD:(ch + 1) * LPAD],
                            start=(h == 0 and dk == 0),
                            stop=(h == 1 and dk == DKT6 // 2 - 1),
                            perf_mode=DR,
                        )
                if h == 0:
                    continue
                sl = slice(bt * LPAD, (bt + 1) * LPAD)
                tg = f"s384_{bt}"
                q1 = scr.tile([128, LPAD], F32, tag=tg, name=f"l1_{bt}")
                nc.scalar.activation(q1[:], ps2[bt][0][:, :LPAD], ACTF.Square,
                                     scale=INV_W)
                q2 = scr.tile([128, LPAD], F32, tag=tg, name=f"l2_{bt}")
                nc.scalar.activation(q2[:], ps2[bt][1][:, :LPAD], ACTF.Square,
                                     scale=INV_W)
                imsq = scr.tile([128, LPAD], F32, tag=tg, name=f"l3_{bt}")
                nc.vector.tensor_tensor(out=imsq[:], in0=q1[:], in1=q2[:],
                                        op=ALU.add)
                dl = scr.tile([128, LPAD], F32, tag=tg, name=f"l4_{bt}")
                nc.vector.tensor_tensor(out=dl[:], in0=imsq[:],
                                        in1=small["l2r"][:], op=ALU.subtract)
                rl = scr.tile([128, LPAD], F32, tag=tg, name=f"l5_{bt}")
                nc.vector.tensor_scalar(out=rl[:], in0=dl[:], scalar1=0.0,
                                        scalar2=None, op0=ALU.max,
                                        op1=ALU.add, accum_out=slot_p(bt))
                ml = scr.tile([128, LPAD], F32, tag=tg, name=f"l6_{bt}")
                nc.vector.tensor_tensor(out=ml[:], in0=dl[:],
                                        in1=small["miu"][:, sl], op=ALU.mult)
                al = scr.tile([128, LPAD], F32, tag=tg, name=f"l7_{bt}")
                nc.scalar.activation(al[:], ml[:], ACTF.Abs,
                                     accum_out=slot_p(bt))

            # ---- final per-batch reduction and output (single [128, 2]
            # store; host reorders)
            outsb = res.tile([128, 2], F32)
            for bt in range(2):
                rp = scr.tile([128, 1], F32, tag="s1", name=f"rp{bt}")
                nc.vector.reduce_sum(out=rp[:], in_=accp[:, bt, :],
                                     axis=mybir.AxisListType.X)
                rn = scr.tile([128, 1], F32, tag="s1", name=f"rn{bt}")
                nc.vector.reduce_sum(out=rn[:], in_=accn[:, bt, :],
                                     axis=mybir.AxisListType.X)
                nc.vector.tensor_tensor(out=outsb[:, bt:bt + 1], in0=rp[:],
                                        in1=rn[:], op=ALU.subtract)
            nc.scalar.dma_start(d_out[:], outsb[:])

    nc.compile()
    return nc


# ---------------------------------------------------------------- host prep
def _ktile(wt, kt_n, c):
    """[K, C] -> [128, kt_n*C] with column block per k-tile."""
    return np.ascontiguousarray(
        wt.reshape(kt_n, 128, c).transpose(1, 0, 2).reshape(128, kt_n * c))


def _btile(a):
    """[256, F] -> [128, 2F] with b-tile column blocks."""
    return np.ascontiguousarray(np.concatenate([a[:128], a[128:]], axis=1))


def _f8(a):
    return np.asarray(a).astype(ml_dtypes.float8_e4m3)


def _bf(a):
    return a.astype(ml_dtypes.bfloat16)


def _prep(inp):
    f32 = np.float32
    Volt = np.asarray(inp["Volt"], f32)
    S = np.asarray(inp["Y"], f32) + np.asarray(inp["Yconj"], f32)
    IM = np.asarray(inp["IM"], f32)
    Ybr = np.asarray(inp["Ybr"], f32)
    Map_g = np.asarray(inp["Map_g"], f32)
    nolp = np.asarray(inp["n_o_l_p"], f32)
    Lg = np.asarray(inp["Lg_Max"], f32)
    PQG = np.asarray(inp["PQ_Gens"], f32)
    PQL = np.asarray(inp["PQ_Loads"], f32)
    mgu = np.asarray(inp["n_o_mu_g_u"], f32)
    mgd = np.asarray(inp["n_o_mu_g_d"], f32)
    mvu = np.asarray(inp["n_o_mu_v_u"], f32)
    mvd = np.asarray(inp["n_o_mu_v_d"], f32)
    miu = np.asarray(inp["n_o_mu_i_u"], f32)
    gmax = np.asarray(inp["Gen_max"], f32)
    gmin = np.asarray(inp["Gen_min"], f32)
    vmax = np.asarray(inp["V_max"], f32)
    vmin = np.asarray(inp["V_min"], f32)
    llim = np.asarray(inp["L_limit"], f32)
    cpg = np.asarray(inp["C_Pg"], f32)
    cqg = np.asarray(inp["C_Qg"], f32)
    n_gbus = int(inp["n_gbus"])
    slack = int(inp["slack_bus_idx"])

    n2 = 2 * N
    K4 = KT4 * 128
    K6 = KT6 * 128
    sV_hi = Volt[:, N:n2].sum(1, dtype=np.float64).astype(f32)
    cpq_full = np.concatenate([cpg, cqg], axis=1)

    # shared across cores: activations scaled by SA
    vp = np.zeros((K4, 256), f32)
    vp[:n2] = Volt.T * SA
    vt_full = _f8(_ktile(vp, KT4, 256))
    ap_ = np.zeros((K4, 256), f32)
    ap_[:n2] = (nolp * (Lg[0] * SA)).T
    at_full = _f8(_ktile(ap_, KT4, 256))

    in_maps = []
    for c in range(NCORE):
        iY = slice(YROW * c, YROW * (c + 1))
        iT = slice(TROW * c, TROW * (c + 1))
        iM = slice(MROW * c, MROW * (c + 1))
        iL = slice(LROW * c, LROW * (c + 1))
        iV = slice(VROW * c, VROW * (c + 1))

        z = np.zeros((K4, 256), f32)
        z[:n2, 0:YROW] = S[iY, :].T * SW
        z[:n2, YROW] = S[N + 1, :] * SW
        yy_c = _f8(_ktile(z, KT4, 256))

        z = np.zeros((K4, TPAD), f32)
        z[:n2, :TROW] = IM[iT, :].T * SW
        imt_c = _f8(_ktile(z, KT4, TPAD))

        z = np.zeros((K4, MPAD), f32)
        z[:n2, :MROW] = Map_g[iM, :].T * SW
        mapt_c = _f8(_ktile(z, KT4, MPAD))

        # gathered-T row order: half A (t 0-383 of each core, core-major),
        # then half B (t 384-767, the tail 750-767 zero-padded)
        z = np.zeros((K6, TPAD), f32)
        rr = slice(LROW * c, LROW * (c + 1))
        ri = slice(NL + LROW * c, NL + LROW * (c + 1))
        HB = K6 // 2
        for blk in range(NCORE):
            ta = slice(TROW * blk, TROW * blk + 384)
            za = slice(blk * 384, blk * 384 + 384)
            z[za, 0:LROW] = Ybr[rr, ta].T * SW
            z[za, LPAD:LPAD + LROW] = Ybr[ri, ta].T * SW
            tb = slice(TROW * blk + 384, TROW * (blk + 1))
            zb = slice(HB + blk * 384, HB + blk * 384 + (TROW - 384))
            z[zb, 0:LROW] = Ybr[rr, tb].T * SW
            z[zb, LPAD:LPAD + LROW] = Ybr[ri, tb].T * SW
        ybrt_c = _f8(_ktile(z, KT6, TPAD))

        # quadratic-term multiplier, pre-divided by SA*SW
        m = np.zeros((256, 256), f32)
        m[:, 0:YROW] = Volt[:, iY] * INV_AW
        m[:, YROW] = sV_hi * (INV_AW / NCORE)

        def padw(a, w):
            z = np.zeros((256, w), f32)
            z[:, :a.shape[1]] = a
            return z

        def repl(vec, w, pad):
            r = np.full(w, pad, f32)
            r[:vec.shape[0]] = vec
            return np.broadcast_to(r, (128, w))

        parts = {
            "mult": _btile(m),
            "pqg": _btile(padw(PQG[:, iM], 512)),
            "mgu": _btile(padw(mgu[:, iM], 512)),
            "mgd": _btile(padw(mgd[:, iM], 512)),
            "cpq": _btile(padw(cpq_full[:, iM] * (SA * SW), 512)),
            "vr": _btile(padw(Volt[:, iV], VPAD)),
            "vi": _btile(padw(Volt[:, N + VROW * c: N + VROW * (c + 1)],
                              VPAD)),
            "mvu": _btile(padw(mvu[:, iV], VPAD)),
            "mvd": _btile(padw(mvd[:, iV], VPAD)),
            "miu": _btile(padw(miu[:, iL], LPAD)),
            "gmaxr": repl(gmax[iM], 512, 1.0),
            "gminr": repl(gmin[iM], 512, -1.0),
            "vmax2r": repl(vmax[iV] ** 2, VPAD, 1.0),
            "vmin2r": repl(vmin[iV] ** 2, VPAD, -1.0),
            "l2r": repl(llim[iL] ** 2, LPAD, 1.0),
        }
        blob = np.zeros((128, _BLOBW), ml_dtypes.bfloat16)
        for nm, (o, w) in _BLOB_OFF.items():
            blob[:, o:o + w] = _bf(np.ascontiguousarray(parts[nm]))

        cols_c = np.broadcast_to(
            np.array([Lg[1] * SA * SW, Lg[2] * SA * SW, 1.0 / n_gbus], f32),
            (128, 3)).copy()

        in_maps.append({
            "vt": vt_full, "at": at_full, "yy": yy_c, "imt": imt_c,
            "mapt": mapt_c, "ybrt": ybrt_c, "blob": blob, "cols": cols_c,
        })

    # host-side tiny terms: slack voltage + pq sums
    h0 = (np.abs(Volt[:, slack]).astype(np.float64)
          + (PQL.astype(np.float64) - PQG.astype(np.float64)).sum(1))
    return in_maps, h0.astype(f32)


# ---------------------------------------------------------------- entry
def kernel(**inputs):
    if "nc" not in _CACHE:
        _CACHE["nc"] = _build_nc()
    nc = _CACHE["nc"]
    in_maps, h0 = _prep(inputs)
    res = run_bass_kernel_spmd(
        nc, in_maps, core_ids=list(range(NCORE)),
        trace=bool(int(os.environ.get("KKT_TRACE", "0"))),
    )
    _CACHE["last_exec_time_ns"] = res.exec_time_ns
    total = h0.astype(np.float64)
    for r in res.results:
        o = r["out"].astype(np.float64)
        total = total + np.concatenate([o[:, 0], o[:, 1]])
    return total.astype(np.float32)


# revision 13
# speedup vs baseline: 1.1066x; 1.1066x over previous
"""Trainium2 Bass kernel for the KKT loss (nn_KKTLoss_46299747451217).

Strategy (8 NeuronCores, SPMD), v2 — fp8 DoubleRow:
  - All matmul operands are fp8 e4m3 (PE DoubleRow mode: 2 contraction rows
    per cycle, fp32 PSUM accumulation). Power-of-2 scales keep values in the
    e4m3 normal range: activations x4, grid matrices x64; the compensation
    (exact powers of two) is folded into the element-wise multipliers and
    activation scale factors.
  - Y and Yconj are folded into S = Y + Yconj host-side (the reference only
    ever uses Y-rows and Yconj-rows through the same quadratic form).
  - Row-sharding as v1: S 250 rows/core (+ row n+1), IM 750 rows/core,
    Ybr 375 real + 375 imag line rows/core, Map_g 500 rows/core.
  - Stage 1 computes T^T directly ([t-rows, batch] PSUM tiles, IM-stationary
    matmuls) so no PE transposes are needed; T^T is stored fp8 and
    AllGathered (1.5 MB instead of 3.1 MB).
  - DMA priority: the sync queue carries only the stage-1-critical vt/imt
    chunks; the scalar queue (blob + S/at/mapt) and vector queue (ybrt
    prefetch) are gated on early imt chunks so they don't steal HBM
    bandwidth from stage 1. The ttg read-back is queued on gpsimd directly
    behind the collective.
  - Element-wise penalties in bf16 exactly as v1 (proven 5e-4 accurate);
    fp8 matmul error verified ~1e-2 end-to-end vs the fp32 reference.
  - Each core outputs a partial [256] loss; the host sums the 8 partials and
    adds the tiny slack/pq terms.
"""

import os
import numpy as np
import ml_dtypes

import concourse.bass as bass
import concourse.bacc as bacc
import concourse.mybir as mybir
import concourse.tile as tile
from concourse.bass_utils import run_bass_kernel_spmd

F32 = mybir.dt.float32
BF16 = mybir.dt.bfloat16
FP8 = mybir.dt.float8e4
ALU = mybir.AluOpType
ACTF = mybir.ActivationFunctionType
DR = mybir.MatmulPerfMode.DoubleRow

# ---------------------------------------------------------------- constants
B = 256            # batch
N = 2000           # n_bus
NL = 3000          # n_line
NCORE = 8
KT4, DKT4 = 32, 16   # k tiles / double-k tiles over padded 2n = 4096
KT6, DKT6 = 48, 24   # k tiles over gathered-T contraction (8 * 768)
YROW = 250         # S rows per core
TROW, TPAD = 750, 768
MROW, MPAD = 500, 512
LROW, LPAD = 375, 384
VROW, VPAD = 250, 256
NPs = 12           # positive accumulator slots per b-tile
NNs = 8            # negative accumulator slots per b-tile

SA = 4.0           # activation fp8 scale (Volt, n_o_l_p*Lg0)
SW = 64.0          # matrix fp8 scale (S, IM, Ybr, Map_g)
# stage1 psum = SA*SW*T -> store T*1; stage2 psum = SW*Ibr; Y psum = SA*SW*SV
INV_AW = 1.0 / (SA * SW)   # 2^-8
INV_W = 1.0 / SW           # 2^-6

# blob layout: name -> (offset, width), all bf16, [128, _BLOBW]
_BLOB_SPEC = [
    ("mult", 512), ("pqg", 1024), ("mgu", 1024), ("mgd", 1024),
    ("cpq", 1024), ("vr", 512), ("vi", 512), ("mvu", 512), ("mvd", 512),
    ("miu", 768), ("gmaxr", 512), ("gminr", 512), ("vmax2r", 256),
    ("vmin2r", 256), ("l2r", 384),
]
_BLOB_OFF = {}
_off = 0
for _nm, _w in _BLOB_SPEC:
    _BLOB_OFF[_nm] = (_off, _w)
    _off += _w
_BLOBW = _off

_CACHE = {}


# ---------------------------------------------------------------- builders
def _build_nc():
    nc = bacc.Bacc("TRN2", target_bir_lowering=False, debug=False,
                   num_devices=NCORE)

    # fp8 k-tile-packed matrices: [128, KT*C] with column block per k-tile
    d_vt = nc.dram_tensor("vt", [128, KT4 * 256], FP8, kind="ExternalInput")
    d_at = nc.dram_tensor("at", [128, KT4 * 256], FP8, kind="ExternalInput")
    d_yy = nc.dram_tensor("yy", [128, KT4 * 256], FP8, kind="ExternalInput")
    d_imt = nc.dram_tensor("imt", [128, KT4 * TPAD], FP8, kind="ExternalInput")
    d_mapt = nc.dram_tensor("mapt", [128, KT4 * MPAD], FP8,
                            kind="ExternalInput")
    d_ybrt = nc.dram_tensor("ybrt", [128, KT6 * TPAD], FP8,
                            kind="ExternalInput")
    d_blob = nc.dram_tensor("blob", [128, _BLOBW], BF16, kind="ExternalInput")
    # per-partition scalar columns: [256*Lg1, 256*Lg2, 1/n_gbus]
    d_cols = nc.dram_tensor("cols", [128, 3], F32, kind="ExternalInput")
    d_out = nc.dram_tensor("out", [128, 2], F32, kind="ExternalOutput")

    with tile.TileContext(nc) as tc:
        with (
            tc.tile_pool(name="res", bufs=1) as res,
            tc.tile_pool(name="scr", bufs=4) as scr,
            tc.tile_pool(name="ps", bufs=8, space="PSUM") as ps,
            tc.tile_pool(name="dram", bufs=1, space="DRAM") as dram,
        ):
            # ---- stage-1-critical loads on the sync queue, interleaved so
            # the first double-k-tile matmuls unblock after ~1 chunk
            vt = res.tile([128, KT4, 256], FP8)
            imt = res.tile([128, KT4, TPAD], FP8)
            vt2 = vt.rearrange("p k c -> p (k c)")
            imt2 = imt.rearrange("p k c -> p (k c)")
            for j in range(8):
                if j % 2 == 0:
                    jv = j // 2
                    nc.sync.dma_start(vt2[:, jv * 2048:(jv + 1) * 2048],
                                      d_vt[:, jv * 2048:(jv + 1) * 2048])
                # spread imt over the sync and gpsimd rings for more DMA
                # engine coverage in the critical first microseconds
                eng = nc.sync if j % 2 == 0 else nc.gpsimd
                eng.dma_start(
                    imt2[:, j * 4 * TPAD:(j + 1) * 4 * TPAD],
                    d_imt[:, j * 4 * TPAD:(j + 1) * 4 * TPAD])

            # ---- scalar queue: gate on a late imt chunk so the stage-1
            # loads get the full HBM bandwidth first (ring descriptors
            # execute concurrently, so everything here is held back by the
            # gate), then load the Y/Map/penalty inputs and prefetch ybrt
            gate_s = res.tile([128, 2], F32)
            nc.scalar.activation(gate_s[0:1, 0:1],
                                 imt2[0:1, 23 * TPAD + 767:23 * TPAD + 768],
                                 ACTF.Copy)
            yy = res.tile([128, KT4, 256], FP8)
            nc.scalar.dma_start(yy.rearrange("p k c -> p (k c)")[:], d_yy[:])
            at = res.tile([128, KT4, 256], FP8)
            nc.scalar.dma_start(at.rearrange("p k c -> p (k c)")[:], d_at[:])
            mapt = res.tile([128, KT4, MPAD], FP8)
            mapt2 = mapt.rearrange("p k c -> p (k c)")
            nc.scalar.dma_start(mapt2[:, :KT4 * MPAD // 2],
                                d_mapt[:, :KT4 * MPAD // 2])
            nc.scalar.dma_start(mapt2[:, KT4 * MPAD // 2:],
                                d_mapt[:, KT4 * MPAD // 2:])
            blob = res.tile([128, _BLOBW], BF16)
            blob_h = _BLOBW // 2
            nc.scalar.dma_start(blob[:, :blob_h], d_blob[:, :blob_h])
            nc.scalar.dma_start(blob[:, blob_h:], d_blob[:, blob_h:])
            cols = res.tile([128, 3], F32)
            nc.scalar.dma_start(cols[:], d_cols[:])
            ybr = res.tile([128, KT6, TPAD], FP8)
            ybr2 = ybr.rearrange("p k c -> p (k c)")
            for j in range(6):
                nc.scalar.dma_start(
                    ybr2[:, j * 8 * TPAD:(j + 1) * 8 * TPAD],
                    d_ybrt[:, j * 8 * TPAD:(j + 1) * 8 * TPAD])

            small = {nm: blob[:, o:o + w] for nm, (o, w) in _BLOB_OFF.items()}
            lg1 = cols[:, 0:1]
            lg2 = cols[:, 1:2]
            ngbinv = cols[:, 2:3]

            # ---- PE warm-up: dummy matmuls on a zeroed tile ramp the tensor
            # engine's p-state to full clock while the stage-1 DMAs land
            warm = res.tile([128, 2, 256], FP8)
            nc.vector.memset(warm.rearrange("p a b -> p (a b)")[:], 0.0)
            wps = ps.tile([128, 512], F32, tag="mm", name="warm_ps")
            for _ in range(24):
                nc.tensor.matmul(
                    wps[:, :256], warm[:, :, 0:128], warm[:],
                    start=True, stop=True, perf_mode=DR,
                    skip_group_check=True,
                )

            # ---- stage 1: T^T tiles [t(128), batch(256)], IM-stationary.
            # Half A (t-blocks 0-2) runs its FULL contraction first so
            # collective A can launch ~25us earlier; half B computes while
            # gather A is in flight.
            ps1 = [ps.tile([128, 512], F32, tag="mm", name=f"ps1_{tb}")
                   for tb in range(6)]
            tt_dram = [dram.tile([3 * 128, 256], FP8, name=f"tt{h}")
                       for h in range(2)]
            ttg_dram = [dram.tile([24 * 128, 256], FP8, addr_space="Shared",
                                  name=f"ttg{h}") for h in range(2)]
            tsb = [scr.tile([128, 3, 256], FP8, tag="tsb", name=f"tsb{h}")
                   for h in range(2)]
            for h in range(2):
                for dk in range(DKT4):
                    for tb in range(3 * h, 3 * h + 3):
                        nc.tensor.matmul(
                            ps1[tb][:, :256],
                            imt[:, 2 * dk:2 * dk + 2,
                                tb * 128:(tb + 1) * 128],
                            vt[:, 2 * dk:2 * dk + 2, :],
                            start=(dk == 0), stop=(dk == DKT4 - 1),
                            perf_mode=DR,
                        )
                # drain to fp8 (T_true = psum / (SA*SW)) and store as one
                # descriptor; A rides gpsimd (in front of the collectives),
                # B rides the idle sync ring
                for tb in range(3):
                    nc.scalar.activation(tsb[h][:, tb, :],
                                         ps1[3 * h + tb][:, :256], ACTF.Copy,
                                         scale=INV_AW)
                tdv = tt_dram[h].rearrange("(t p) b -> p t b", p=128)
                if h == 0:
                    nc.gpsimd.dma_start(tdv, tsb[h][:])
                else:
                    nc.sync.dma_start(tdv, tsb[h][:])
            for h in range(2):
                nc.gpsimd.collective_compute(
                    "AllGather",
                    ALU.bypass,
                    replica_groups=[list(range(NCORE))],
                    ins=[tt_dram[h].opt()],
                    outs=[ttg_dram[h].opt()],
                )
            # read back the gathered T^T halves, k-tiled, on the scalar ring
            # (data-dependent on each collective; overlaps the next one)
            ttg = res.tile([128, KT6, 256], FP8)
            for h in range(2):
                for j in range(2):
                    tv = ttg_dram[h][j * 1536:(j + 1) * 1536, :].rearrange(
                        "(k p) b -> p k b", p=128)
                    nc.scalar.dma_start(
                        ttg[:, h * 24 + j * 12:h * 24 + (j + 1) * 12, :], tv)

            # accumulator strips
            accp = res.tile([128, 2, NPs], F32)
            accn = res.tile([128, 2, NNs], F32)
            nc.vector.memset(accp[:], 0.0)
            nc.vector.memset(accn[:], 0.0)
            ip = [0, 0]
            iq = [0, 0]

            def slot_p(bt):
                j = ip[bt]
                ip[bt] += 1
                assert j < NPs
                return accp[:, bt, j:j + 1]

            def slot_n(bt):
                j = iq[bt]
                iq[bt] += 1
                assert j < NNs
                return accn[:, bt, j:j + 1]

            # ---- S = Y+Yconj quadratic term (psum = SA*SW*(S V);
            # mult blob is pre-divided by SA*SW)
            for bt in range(2):
                q = ps.tile([128, 512], F32, tag="mm", name=f"q{bt}")
                for dk in range(DKT4):
                    nc.tensor.matmul(
                        q[:, :256],
                        vt[:, 2 * dk:2 * dk + 2, bt * 128:(bt + 1) * 128],
                        yy[:, 2 * dk:2 * dk + 2, :],
                        start=(dk == 0), stop=(dk == DKT4 - 1),
                        perf_mode=DR,
                    )
                oq = scr.tile([128, 256], F32, tag="s256y", name=f"oq{bt}")
                nc.vector.tensor_tensor(
                    out=oq[:], in0=q[:, :256],
                    in1=small["mult"][:, bt * 256:(bt + 1) * 256], op=ALU.mult)
                nc.vector.reduce_sum(out=slot_p(bt), in_=oq[:],
                                     axis=mybir.AxisListType.X)

            # ---- Map_g dual/stationarity term (psum = SA*SW*(a Map^T);
            # lg1/lg2/cpq are pre-scaled by SA*SW, final Abs scales back)
            for bt in range(2):
                d = ps.tile([128, 512], F32, tag="mm", name=f"d{bt}")
                for dk in range(DKT4):
                    nc.tensor.matmul(
                        d[:],
                        at[:, 2 * dk:2 * dk + 2, bt * 128:(bt + 1) * 128],
                        mapt[:, 2 * dk:2 * dk + 2, :],
                        start=(dk == 0), stop=(dk == DKT4 - 1),
                        perf_mode=DR,
                    )
                sl = slice(bt * 512, (bt + 1) * 512)
                t1 = scr.tile([128, 512], F32, tag="s512", name=f"du1_{bt}")
                nc.vector.scalar_tensor_tensor(
                    out=t1[:], in0=small["mgu"][:, sl], scalar=lg1, in1=d[:],
                    op0=ALU.mult, op1=ALU.add)
                t2 = scr.tile([128, 512], F32, tag="s512", name=f"du2_{bt}")
                nc.vector.scalar_tensor_tensor(
                    out=t2[:], in0=small["mgd"][:, sl], scalar=lg2, in1=t1[:],
                    op0=ALU.mult, op1=ALU.subtract)
                t3 = scr.tile([128, 512], F32, tag="s512", name=f"du3_{bt}")
                nc.vector.tensor_tensor(
                    out=t3[:], in0=t2[:], in1=small["cpq"][:, sl], op=ALU.add)
                t4 = scr.tile([128, 512], F32, tag="s512", name=f"du4_{bt}")
                nc.scalar.activation(t4[:], t3[:], ACTF.Abs, scale=INV_AW,
                                     accum_out=slot_p(bt))

            # ---- generator limit + complementary slackness terms
            for bt in range(2):
                sl = slice(bt * 512, (bt + 1) * 512)
                d1 = scr.tile([128, 512], F32, tag="s512", name=f"g1_{bt}")
                nc.vector.tensor_tensor(out=d1[:], in0=small["pqg"][:, sl],
                                        in1=small["gmaxr"][:], op=ALU.subtract)
                r1 = scr.tile([128, 512], F32, tag="s512", name=f"g2_{bt}")
                nc.vector.tensor_scalar(out=r1[:], in0=d1[:], scalar1=0.0,
                                        scalar2=None, op0=ALU.max,
                                        op1=ALU.add, accum_out=slot_p(bt))
                m1 = scr.tile([128, 512], F32, tag="s512", name=f"g3_{bt}")
                nc.vector.tensor_tensor(out=m1[:], in0=d1[:],
                                        in1=small["mgu"][:, sl], op=ALU.mult)
                a1 = scr.tile([128, 512], F32, tag="s512", name=f"g4_{bt}")
                nc.scalar.activation(a1[:], m1[:], ACTF.Abs, scale=ngbinv,
                                     accum_out=slot_p(bt))

                d2 = scr.tile([128, 512], F32, tag="s512", name=f"g5_{bt}")
                nc.vector.tensor_tensor(out=d2[:], in0=small["pqg"][:, sl],
                                        in1=small["gminr"][:], op=ALU.subtract)
                r2 = scr.tile([128, 512], F32, tag="s512", name=f"g6_{bt}")
                nc.vector.tensor_scalar(out=r2[:], in0=d2[:], scalar1=0.0,
                                        scalar2=None, op0=ALU.min,
                                        op1=ALU.add, accum_out=slot_n(bt))
                m2 = scr.tile([128, 512], F32, tag="s512", name=f"g7_{bt}")
                nc.vector.tensor_tensor(out=m2[:], in0=d2[:],
                                        in1=small["mgd"][:, sl], op=ALU.mult)
                a2 = scr.tile([128, 512], F32, tag="s512", name=f"g8_{bt}")
                nc.scalar.activation(a2[:], m2[:], ACTF.Abs, scale=ngbinv,
                                     accum_out=slot_p(bt))

            # ---- voltage magnitude terms
            for bt in range(2):
                sl = slice(bt * VPAD, (bt + 1) * VPAD)
                s1 = scr.tile([128, VPAD], F32, tag="s256", name=f"v1_{bt}")
                nc.scalar.activation(s1[:], small["vr"][:, sl], ACTF.Square)
                s2 = scr.tile([128, VPAD], F32, tag="s256", name=f"v2_{bt}")
                nc.scalar.activation(s2[:], small["vi"][:, sl], ACTF.Square)
                msq = scr.tile([128, VPAD], F32, tag="s256", name=f"v3_{bt}")
                nc.vector.tensor_tensor(out=msq[:], in0=s1[:], in1=s2[:],
                                        op=ALU.add)
                dv1 = scr.tile([128, VPAD], F32, tag="s256", name=f"v4_{bt}")
                nc.vector.tensor_tensor(out=dv1[:], in0=msq[:],
                                        in1=small["vmax2r"][:], op=ALU.subtract)
                rv1 = scr.tile([128, VPAD], F32, tag="s256", name=f"v5_{bt}")
                nc.vector.tensor_scalar(out=rv1[:], in0=dv1[:], scalar1=0.0,
                                        scalar2=None, op0=ALU.max,
                                        op1=ALU.add, accum_out=slot_p(bt))
                mv1 = scr.tile([128, VPAD], F32, tag="s256", name=f"v6_{bt}")
                nc.vector.tensor_tensor(out=mv1[:], in0=dv1[:],
                                        in1=small["mvu"][:, sl], op=ALU.mult)
                av1 = scr.tile([128, VPAD], F32, tag="s256", name=f"v7_{bt}")
                nc.scalar.activation(av1[:], mv1[:], ACTF.Abs,
                                     accum_out=slot_p(bt))
                dv2 = scr.tile([128, VPAD], F32, tag="s256", name=f"v8_{bt}")
                nc.vector.tensor_tensor(out=dv2[:], in0=msq[:],
                                        in1=small["vmin2r"][:], op=ALU.subtract)
                rv2 = scr.tile([128, VPAD], F32, tag="s256", name=f"v9_{bt}")
                nc.vector.tensor_scalar(out=rv2[:], in0=dv2[:], scalar1=0.0,
                                        scalar2=None, op0=ALU.min,
                                        op1=ALU.add, accum_out=slot_n(bt))
                mv2 = scr.tile([128, VPAD], F32, tag="s256", name=f"va_{bt}")
                nc.vector.tensor_tensor(out=mv2[:], in0=dv2[:],
                                        in1=small["mvd"][:, sl], op=ALU.mult)
                av2 = scr.tile([128, VPAD], F32, tag="s256", name=f"vb_{bt}")
                nc.scalar.activation(av2[:], mv2[:], ACTF.Abs,
                                     accum_out=slot_p(bt))

            # ---- dual feasibility: sum relu(-mu) == -sum min(mu, 0)
            for bt in range(2):
                for nm, w in (("mgu", 512), ("mgd", 512), ("mvu", VPAD),
                              ("mvd", VPAD), ("miu", LPAD)):
                    sl = slice(bt * w, (bt + 1) * w)
                    f = scr.tile([128, w], F32, tag=f"s{w}",
                                 name=f"f_{nm}_{bt}")
                    nc.vector.tensor_scalar(out=f[:], in0=small[nm][:, sl],
                                            scalar1=0.0, scalar2=None,
                                            op0=ALU.min, op1=ALU.add,
                                            accum_out=slot_n(bt))

            # ---- stage 2: branch currents (psum = SW*Ibr; squares are
            # rescaled by 1/SW inside the Square activation). Phase order
            # bt0-A, bt1-A, bt0-B, bt1-B keeps the tensor engine busy on
            # A-half matmuls while gather B is still in flight, and lets
            # bt0's penalty chain overlap bt1's B-half matmuls.
            ps2 = [[ps.tile([128, 512], F32, name=f"ps2_{bt}_{ch}", tag="mm")
                    for ch in range(2)] for bt in range(2)]
            for h, bt in ((0, 0), (0, 1), (1, 0), (1, 1)):
                for dk in range(DKT6 // 2):
                    kk = h * 24 + 2 * dk
                    for ch in range(2):
                        nc.tensor.matmul(
                            ps2[bt][ch][:, :LPAD],
                            ttg[:, kk:kk + 2, bt * 128:(bt + 1) * 128],
                            ybr[:, kk:kk + 2, ch * LPAD:(ch + 1) * LPAD],
                            start=(h == 0 and dk == 0),
                            stop=(h == 1 and dk == DKT6 // 2 - 1),
                            perf_mode=DR,
                        )
                if h == 0:
                    continue
                sl = slice(bt * LPAD, (bt + 1) * LPAD)
                tg = f"s384_{bt}"
                q1 = scr.tile([128, LPAD], F32, tag=tg, name=f"l1_{bt}")
                nc.scalar.activation(q1[:], ps2[bt][0][:, :LPAD], ACTF.Square,
                                     scale=INV_W)
                q2 = scr.tile([128, LPAD], F32, tag=tg, name=f"l2_{bt}")
                nc.scalar.activation(q2[:], ps2[bt][1][:, :LPAD], ACTF.Square,
                                     scale=INV_W)
                imsq = scr.tile([128, LPAD], F32, tag=tg, name=f"l3_{bt}")
                nc.vector.tensor_tensor(out=imsq[:], in0=q1[:], in1=q2[:],
                                        op=ALU.add)
                dl = scr.tile([128, LPAD], F32, tag=tg, name=f"l4_{bt}")
                nc.vector.tensor_tensor(out=dl[:], in0=imsq[:],
                                        in1=small["l2r"][:], op=ALU.subtract)
                rl = scr.tile([128, LPAD], F32, tag=tg, name=f"l5_{bt}")
                nc.vector.tensor_scalar(out=rl[:], in0=dl[:], scalar1=0.0,
                                        scalar2=None, op0=ALU.max,
                                        op1=ALU.add, accum_out=slot_p(bt))
                ml = scr.tile([128, LPAD], F32, tag=tg, name=f"l6_{bt}")
                nc.vector.tensor_tensor(out=ml[:], in0=dl[:],
                                        in1=small["miu"][:, sl], op=ALU.mult)
                al = scr.tile([128, LPAD], F32, tag=tg, name=f"l7_{bt}")
                nc.scalar.activation(al[:], ml[:], ACTF.Abs,
                                     accum_out=slot_p(bt))

            # ---- final per-batch reduction and output (single [128, 2]
            # store; host reorders)
            outsb = res.tile([128, 2], F32)
            for bt in range(2):
                rp = scr.tile([128, 1], F32, tag="s1", name=f"rp{bt}")
                nc.vector.reduce_sum(out=rp[:], in_=accp[:, bt, :],
                                     axis=mybir.AxisListType.X)
                rn = scr.tile([128, 1], F32, tag="s1", name=f"rn{bt}")
                nc.vector.reduce_sum(out=rn[:], in_=accn[:, bt, :],
                                     axis=mybir.AxisListType.X)
                nc.vector.tensor_tensor(out=outsb[:, bt:bt + 1], in0=rp[:],
                                        in1=rn[:], op=ALU.subtract)
            nc.scalar.dma_start(d_out[:], outsb[:])

    nc.compile()
    return nc


# ---------------------------------------------------------------- host prep
def _ktile(wt, kt_n, c):
    """[K, C] -> [128, kt_n*C] with column block per k-tile."""
    return np.ascontiguousarray(
        wt.reshape(kt_n, 128, c).transpose(1, 0, 2).reshape(128, kt_n * c))


def _btile(a):
    """[256, F] -> [128, 2F] with b-tile column blocks."""
    return np.ascontiguousarray(np.concatenate([a[:128], a[128:]], axis=1))


def _f8(a):
    return np.asarray(a).astype(ml_dtypes.float8_e4m3)


def _bf(a):
    return a.astype(ml_dtypes.bfloat16)


def _prep(inp):
    f32 = np.float32
    Volt = np.asarray(inp["Volt"], f32)
    S = np.asarray(inp["Y"], f32) + np.asarray(inp["Yconj"], f32)
    IM = np.asarray(inp["IM"], f32)
    Ybr = np.asarray(inp["Ybr"], f32)
    Map_g = np.asarray(inp["Map_g"], f32)
    nolp = np.asarray(inp["n_o_l_p"], f32)
    Lg = np.asarray(inp["Lg_Max"], f32)
    PQG = np.asarray(inp["PQ_Gens"], f32)
    PQL = np.asarray(inp["PQ_Loads"], f32)
    mgu = np.asarray(inp["n_o_mu_g_u"], f32)
    mgd = np.asarray(inp["n_o_mu_g_d"], f32)
    mvu = np.asarray(inp["n_o_mu_v_u"], f32)
    mvd = np.asarray(inp["n_o_mu_v_d"], f32)
    miu = np.asarray(inp["n_o_mu_i_u"], f32)
    gmax = np.asarray(inp["Gen_max"], f32)
    gmin = np.asarray(inp["Gen_min"], f32)
    vmax = np.asarray(inp["V_max"], f32)
    vmin = np.asarray(inp["V_min"], f32)
    llim = np.asarray(inp["L_limit"], f32)
    cpg = np.asarray(inp["C_Pg"], f32)
    cqg = np.asarray(inp["C_Qg"], f32)
    n_gbus = int(inp["n_gbus"])
    slack = int(inp["slack_bus_idx"])

    n2 = 2 * N
    K4 = KT4 * 128
    K6 = KT6 * 128
    sV_hi = Volt[:, N:n2].sum(1, dtype=np.float64).astype(f32)
    cpq_full = np.concatenate([cpg, cqg], axis=1)

    # shared across cores: activations scaled by SA
    vp = np.zeros((K4, 256), f32)
    vp[:n2] = Volt.T * SA
    vt_full = _f8(_ktile(vp, KT4, 256))
    ap_ = np.zeros((K4, 256), f32)
    ap_[:n2] = (nolp * (Lg[0] * SA)).T
    at_full = _f8(_ktile(ap_, KT4, 256))

    in_maps = []
    for c in range(NCORE):
        iY = slice(YROW * c, YROW * (c + 1))
        iT = slice(TROW * c, TROW * (c + 1))
        iM = slice(MROW * c, MROW * (c + 1))
        iL = slice(LROW * c, LROW * (c + 1))
        iV = slice(VROW * c, VROW * (c + 1))

        z = np.zeros((K4, 256), f32)
        z[:n2, 0:YROW] = S[iY, :].T * SW
        z[:n2, YROW] = S[N + 1, :] * SW
        yy_c = _f8(_ktile(z, KT4, 256))

        z = np.zeros((K4, TPAD), f32)
        z[:n2, :TROW] = IM[iT, :].T * SW
        imt_c = _f8(_ktile(z, KT4, TPAD))

        z = np.zeros((K4, MPAD), f32)
        z[:n2, :MROW] = Map_g[iM, :].T * SW
        mapt_c = _f8(_ktile(z, KT4, MPAD))

        # gathered-T row order: half A (t 0-383 of each core, core-major),
        # then half B (t 384-767, the tail 750-767 zero-padded)
        z = np.zeros((K6, TPAD), f32)
        rr = slice(LROW * c, LROW * (c + 1))
        ri = slice(NL + LROW * c, NL + LROW * (c + 1))
        HB = K6 // 2
        for blk in range(NCORE):
            ta = slice(TROW * blk, TROW * blk + 384)
            za = slice(blk * 384, blk * 384 + 384)
            z[za, 0:LROW] = Ybr[rr, ta].T * SW
            z[za, LPAD:LPAD + LROW] = Ybr[ri, ta].T * SW
            tb = slice(TROW * blk + 384, TROW * (blk + 1))
            zb = slice(HB + blk * 384, HB + blk * 384 + (TROW - 384))
            z[zb, 0:LROW] = Ybr[rr, tb].T * SW
            z[zb, LPAD:LPAD + LROW] = Ybr[ri, tb].T * SW
        ybrt_c = _f8(_ktile(z, KT6, TPAD))

        # quadratic-term multiplier, pre-divided by SA*SW
        m = np.zeros((256, 256), f32)
        m[:, 0:YROW] = Volt[:, iY] * INV_AW
        m[:, YROW] = sV_hi * (INV_AW / NCORE)

        def padw(a, w):
            z = np.zeros((256, w), f32)
            z[:, :a.shape[1]] = a
            return z

        def repl(vec, w, pad):
            r = np.full(w, pad, f32)
            r[:vec.shape[0]] = vec
            return np.broadcast_to(r, (128, w))

        parts = {
            "mult": _btile(m),
            "pqg": _btile(padw(PQG[:, iM], 512)),
            "mgu": _btile(padw(mgu[:, iM], 512)),
            "mgd": _btile(padw(mgd[:, iM], 512)),
            "cpq": _btile(padw(cpq_full[:, iM] * (SA * SW), 512)),
            "vr": _btile(padw(Volt[:, iV], VPAD)),
            "vi": _btile(padw(Volt[:, N + VROW * c: N + VROW * (c + 1)],
                              VPAD)),
            "mvu": _btile(padw(mvu[:, iV], VPAD)),
            "mvd": _btile(padw(mvd[:, iV], VPAD)),
            "miu": _btile(padw(miu[:, iL], LPAD)),
            "gmaxr": repl(gmax[iM], 512, 1.0),
            "gminr": repl(gmin[iM], 512, -1.0),
            "vmax2r": repl(vmax[iV] ** 2, VPAD, 1.0),
            "vmin2r": repl(vmin[iV] ** 2, VPAD, -1.0),
            "l2r": repl(llim[iL] ** 2, LPAD, 1.0),
        }
        blob = np.zeros((128, _BLOBW), ml_dtypes.bfloat16)
        for nm, (o, w) in _BLOB_OFF.items():
            blob[:, o:o + w] = _bf(np.ascontiguousarray(parts[nm]))

        cols_c = np.broadcast_to(
            np.array([Lg[1] * SA * SW, Lg[2] * SA * SW, 1.0 / n_gbus], f32),
            (128, 3)).copy()

        in_maps.append({
            "vt": vt_full, "at": at_full, "yy": yy_c, "imt": imt_c,
            "mapt": mapt_c, "ybrt": ybrt_c, "blob": blob, "cols": cols_c,
        })

    # host-side tiny terms: slack voltage + pq sums
    h0 = (np.abs(Volt[:, slack]).astype(np.float64)
          + (PQL.astype(np.float64) - PQG.astype(np.float64)).sum(1))
    return in_maps, h0.astype(f32)


# ---------------------------------------------------------------- entry
def kernel(**inputs):
    if "nc" not in _CACHE:
        _CACHE["nc"] = _build_nc()
    nc = _CACHE["nc"]
    in_maps, h0 = _prep(inputs)
    res = run_bass_kernel_spmd(
        nc, in_maps, core_ids=list(range(NCORE)),
        trace=bool(int(os.environ.get("KKT_TRACE", "0"))),
    )
    _CACHE["last_exec_time_ns"] = res.exec_time_ns
    total = h0.astype(np.float64)
    for r in res.results:
        o = r["out"].astype(np.float64)
        total = total + np.concatenate([o[:, 0], o[:, 1]])
    return total.astype(np.float32)
